# revision 1
# baseline (speedup 1.0000x reference)
"""Causal self-attention (RoPE, 16 heads, D=1024, B=2, T=2048) on 8 TRN2 NeuronCores.

Sharding: tensor-parallel over heads — 2 heads per core. Each core computes the
qkv projection for its heads (fp32r matmuls against host-pre-transposed x),
rotary embedding, causal attention in S^T layout (keys on PSUM partitions so
P^T = exp(S^T) feeds the attn@V matmul directly as the moving operand, with a
ones-column in V producing the softmax denominators on the tensor engine),
and a partial output projection against its slice of out_w rows. The host
sums the 8 partial projections and adds out_b. Work is emitted batch-
interleaved (qkv/rope/attention per batch) so the tensor engine stays dense.
"""

import os

import numpy as np

import concourse.mybir as mybir
import concourse.tile as tile
from concourse import bacc
from concourse.bass_utils import run_bass_kernel_spmd

F32 = mybir.dt.float32
F32R = mybir.dt.float32r
EXP = mybir.ActivationFunctionType.Exp

D = 1024
H = 16
HD = 64
B = 2
T = 2048
BT = B * T            # 4096
NCORES = 8
HLOC = H // NCORES    # 2 heads per core
NDC = D // 128        # 8 contraction chunks for the qkv projection
TBW = 512             # qkv token-block width
NTBB = T // TBW       # 4 token blocks per batch
NKB = T // 128        # 16 key blocks per (b, h)
NQ = T // 512         # 4 query super-blocks per (b, h)
SCALE = float(1.0 / np.sqrt(HD))


def build_nc():
    nc = bacc.Bacc("TRN2", debug=False)

    xT = nc.dram_tensor("xT", [D, BT], F32R, kind="ExternalInput")
    w = nc.dram_tensor("w", [D, 3 * HLOC * HD], F32R, kind="ExternalInput")
    ropeP = nc.dram_tensor("ropeP", [128, T], F32, kind="ExternalInput")
    ropeQ = nc.dram_tensor("ropeQ", [128, T], F32, kind="ExternalInput")
    maskb = nc.dram_tensor("maskb", [128, 640], F32R, kind="ExternalInput")
    ident = nc.dram_tensor("ident", [128, 64], F32, kind="ExternalInput")
    onescol = nc.dram_tensor("onescol", [128, NKB], F32R, kind="ExternalInput")
    wo = nc.dram_tensor("wo", [128, D], F32R, kind="ExternalInput")
    y = nc.dram_tensor("y", [BT, D], F32, kind="ExternalOutput")

    with tile.TileContext(nc) as tc:
        with (
            tc.tile_pool(name="const", bufs=1) as const,
            tc.tile_pool(name="big", bufs=1) as big,
            tc.tile_pool(name="xt", bufs=2) as xt_pool,
            tc.tile_pool(name="raw", bufs=3) as raw_pool,
            tc.tile_pool(name="gsw", bufs=1) as gsw_pool,
            tc.tile_pool(name="rtmp", bufs=2) as rtmp_pool,
            tc.tile_pool(name="p_sb", bufs=3) as p_pool,
            tc.tile_pool(name="r_sb", bufs=2) as r_pool,
            tc.tile_pool(name="rb_sb", bufs=2) as rb_pool,
            tc.tile_pool(name="aTb", bufs=2) as aT_pool,
            tc.tile_pool(name="y_sb", bufs=3) as y_pool,
            tc.tile_pool(name="aux_ps", bufs=2, space="PSUM") as aux_psum,
            tc.tile_pool(name="s_ps", bufs=2, space="PSUM") as s_psum,
            tc.tile_pool(name="o_ps", bufs=2, space="PSUM") as o_psum,
        ):
            w_sb = const.tile([128, NDC, 3 * HLOC * HD], F32R)
            nc.sync.dma_start(w_sb[:], w[:].rearrange("(dc p) f -> p dc f", p=128))
            P_sb = const.tile([128, T], F32)
            nc.sync.dma_start(P_sb[:], ropeP[:])
            Q_sb = const.tile([128, T], F32)
            nc.sync.dma_start(Q_sb[:], ropeQ[:])
            mb_sb = const.tile([128, 640], F32R)
            nc.sync.dma_start(mb_sb[:], maskb[:])
            id_sb = const.tile([128, 64], F32)
            nc.sync.dma_start(id_sb[:], ident[:])
            oc_sb = const.tile([128, NKB], F32R)
            nc.sync.dma_start(oc_sb[:], onescol[:])
            wo_sb = const.tile([128, D], F32R)
            nc.sync.dma_start(wo_sb[:], wo[:])

            qrot = big.tile([128, BT], F32R, tag="qrot")
            krot = big.tile([128, BT], F32R, tag="krot")
            vsb = [
                big.tile([128, NKB, 65], F32R, name=f"v{i}", tag=f"v{i}")
                for i in range(B * HLOC)
            ]

            pending_outproj = []

            def emit_outproj_piece(job, i):
                aTb, row_base = job
                row0 = row_base + i * 128
                ysb = y_pool.tile([128, 2, 512], F32, name=f"ysb{row0}", tag="ysb")
                for nb in range(2):
                    yps = aux_psum.tile(
                        [128, 512], F32, name=f"yps{row0}{nb}", tag="aux"
                    )
                    nc.tensor.matmul(
                        yps[:],
                        aTb[:, i * 128:(i + 1) * 128],
                        wo_sb[:, nb * 512:(nb + 1) * 512],
                        start=True,
                        stop=True,
                    )
                    if nb == 0:
                        nc.vector.tensor_copy(ysb[:, nb, :], yps[:])
                    else:
                        nc.scalar.copy(ysb[:, nb, :], yps[:])
                nc.sync.dma_start(
                    y[row0:row0 + 128, :],
                    ysb[:].rearrange("p a t -> p (a t)"),
                )

            def pop_outproj_piece():
                if pending_outproj:
                    job, i = pending_outproj[0]
                    emit_outproj_piece(job, i)
                    if i == 3:
                        pending_outproj.pop(0)
                    else:
                        pending_outproj[0] = (job, i + 1)

            def phase1_thunks(b):
                """Generate emission thunks for batch b's qkv + rope + V-transpose."""
                state = {}

                def get_raws():
                    if "raws" not in state:
                        state["raws"] = [
                            raw_pool.tile(
                                [128, T], F32R, name=f"raw{b}{ft}", tag="raw"
                            )
                            for ft in range(3)
                        ]
                    return state["raws"]

                def qkv_group(tb, ft):
                    raws = get_raws()
                    if ft == 0:
                        col0 = b * T + tb * TBW
                        xt_t = xt_pool.tile(
                            [128, NDC, TBW], F32R, name=f"xt{b}{tb}", tag="xt"
                        )
                        nc.sync.dma_start(
                            xt_t[:],
                            xT[:, col0:col0 + TBW].rearrange(
                                "(dc p) t -> p dc t", p=128
                            ),
                        )
                        state[("xt", tb)] = xt_t
                    xt_t = state[("xt", tb)]
                    ps = aux_psum.tile(
                        [128, TBW], F32, name=f"qkvps{b}{tb}{ft}", tag="aux"
                    )
                    for dc in range(NDC):
                        nc.tensor.matmul(
                            ps[:],
                            w_sb[:, dc, ft * 128:(ft + 1) * 128],
                            xt_t[:, dc, :],
                            start=(dc == 0),
                            stop=(dc == NDC - 1),
                        )
                    dst = raws[ft][:, tb * TBW:(tb + 1) * TBW]
                    if (tb * 3 + ft) % 2 == 0:
                        nc.scalar.copy(dst, ps[:])
                    else:
                        nc.vector.tensor_copy(dst, ps[:])

                def rope(which, tb):
                    raws = get_raws()
                    raw = raws[0] if which == 0 else raws[1]
                    rot = qrot if which == 0 else krot
                    cs = slice(tb * TBW, (tb + 1) * TBW)
                    gsw = gsw_pool.tile(
                        [128, TBW], F32, name=f"gsw{b}{which}{tb}", tag="gsw"
                    )
                    for l in range(HLOC):
                        p0 = l * 64
                        nc.gpsimd.dma_start(
                            gsw[p0:p0 + 32, :], raw[p0 + 32:p0 + 64, cs].bitcast(F32)
                        )
                        nc.gpsimd.dma_start(
                            gsw[p0 + 32:p0 + 64, :], raw[p0:p0 + 32, cs].bitcast(F32)
                        )
                    t1 = rtmp_pool.tile(
                        [128, TBW], F32, name=f"rt{b}{which}{tb}", tag="rt"
                    )
                    nc.vector.tensor_mul(t1[:], raw[:, cs].bitcast(F32), P_sb[:, cs])
                    nc.vector.tensor_mul(gsw[:], gsw[:], Q_sb[:, cs])
                    nc.vector.tensor_add(rot[:, b * T + tb * TBW:b * T + (tb + 1) * TBW],
                                         t1[:], gsw[:])

                def vt_group(l, half):
                    vraw = get_raws()[2]
                    bh = b * HLOC + l
                    if half == 0:
                        nc.gpsimd.dma_start(vsb[bh][:, :, 64], oc_sb[:])
                    for kb in range(half * 8, half * 8 + 8):
                        tp = aux_psum.tile(
                            [128, 64], F32, name=f"tp{b}{l}{kb}", tag="aux"
                        )
                        nc.tensor.transpose(
                            tp[:],
                            vraw[l * 64:(l + 1) * 64,
                                 kb * 128:(kb + 1) * 128].bitcast(F32),
                            id_sb[l * 64:(l + 1) * 64, :],
                        )
                        nc.vector.tensor_copy(vsb[bh][:, kb, 0:64], tp[:])

                thunks = []
                for tb in range(NTBB):
                    for ft in range(3):
                        thunks.append(lambda tb=tb, ft=ft: qkv_group(tb, ft))
                        if ft < 2:
                            thunks.append(lambda which=ft, tb=tb: rope(which, tb))
                for l in range(HLOC):
                    for half in range(2):
                        thunks.append(lambda l=l, half=half: vt_group(l, half))
                return thunks

            def attention(b, filler):
                kb_count = 0
                for qb in range(NQ):
                    q0 = qb * 512
                    nkb = (q0 + 512) // 128
                    opss = [
                        o_psum.tile([65, 512], F32, name=f"ops{b}{qb}{_l}", tag="ops")
                        for _l in range(HLOC)
                    ]

                    def s_pair(kb, _b=b, _qb=qb, _q0=q0):
                        r_off = kb - _qb * 4
                        cm = 128 * r_off if r_off >= 0 else 0
                        k0 = kb * 128
                        ksl = slice(_b * T + k0, _b * T + k0 + 128)
                        qsl = slice(_b * T + _q0 + cm, _b * T + _q0 + 512)
                        sps = s_psum.tile(
                            [128, 2, 512], F32, name=f"sps{_b}{_qb}{kb}", tag="sps",
                        )
                        for l in range(HLOC):
                            p0 = l * 64
                            nc.tensor.matmul(
                                sps[:, l, cm:512],
                                krot[p0:p0 + 64, ksl],
                                qrot[p0:p0 + 64, qsl],
                                start=True,
                                stop=True,
                            )
                        return sps

                    spss = {0: s_pair(0)}
                    for kb in range(nkb):
                        if kb + 1 < nkb:
                            spss[kb + 1] = s_pair(kb + 1)
                        r_off = kb - qb * 4  # >= 0: diagonal-region block
                        pt = p_pool.tile([128, 2, 512], F32R, tag="pt")
                        cm = 128 * r_off if r_off >= 0 else 0
                        sps = spss[kb]
                        if cm == 0:
                            nc.scalar.activation(
                                pt[:].rearrange("p a t -> p (a t)"),
                                sps[:].rearrange("p a t -> p (a t)"),
                                EXP, scale=SCALE,
                            )
                        else:
                            for l in range(HLOC):
                                nc.scalar.activation(
                                    pt[:, l, cm:512], sps[:, l, cm:512],
                                    EXP, scale=SCALE,
                                )
                        if r_off >= 0:
                            for l in range(HLOC):
                                nc.vector.tensor_mul(
                                    pt[:, l, cm:cm + 128],
                                    pt[:, l, cm:cm + 128],
                                    mb_sb[:, 384:512],
                                )
                        for l in range(HLOC):
                            nc.tensor.matmul(
                                opss[l][:, cm:512],
                                vsb[b * HLOC + l][:, kb, :],
                                pt[:, l, cm:512],
                                start=(kb == 0),
                                stop=(kb == nkb - 1),
                            )
                        del spss[kb]
                        pop_outproj_piece()
                        if kb_count % 2 == 1 or kb_count >= 30:
                            f = next(filler, None)
                            if f is not None:
                                f()
                        kb_count += 1
                    aTb = aT_pool.tile([128, 512], F32R, name=f"aTb{b}{qb}", tag="aTb")
                    for l in range(HLOC):
                        ops = opss[l]
                        d_sb = r_pool.tile([1, 512], F32, tag="d")
                        nc.vector.tensor_copy(d_sb[:], ops[64:65, :])
                        r_sb = r_pool.tile([1, 512], F32, tag="r")
                        nc.vector.reciprocal_approx_fast(r_sb[:], d_sb[:])
                        rb_sb = rb_pool.tile([64, 512], F32, tag="rb")
                        nc.gpsimd.partition_broadcast(rb_sb[:], r_sb[:])
                        nc.vector.tensor_mul(
                            aTb[l * 64:(l + 1) * 64, :], ops[0:64, :], rb_sb[:]
                        )
                    pending_outproj.append(((aTb, b * T + q0), 0))

            for th in phase1_thunks(0):
                th()
            p1b1 = iter(phase1_thunks(1))
            attention(0, p1b1)
            for th in p1b1:
                th()
            attention(1, iter(()))

            while pending_outproj:
                job, i = pending_outproj.pop(0)
                for j in range(i, 4):
                    emit_outproj_piece(job, j)

    nc.finalize()
    return nc


def _rope_tables():
    inv_freq = 1.0 / (10000.0 ** (np.arange(0, HD, 2, dtype=np.float32) / HD))
    t = np.arange(T, dtype=np.float32)
    freqs = t[:, None] * inv_freq[None, :]                          # [T, 32]
    rope = np.concatenate([np.sin(freqs), np.cos(freqs)], axis=-1)  # [T, 64]
    sin = rope[:, ::2]    # [T, 32]  (reference's "sin")
    cos = rope[:, 1::2]   # [T, 32]  (reference's "cos")
    # rot = raw * P + swap_halves(raw) * Q  with raw rows [x1(32) ; x2(32)]:
    #  rows 0..31  (out half0 = x1*cos - x2*sin; raw=x1, swap=x2): P=cos, Q=-sin
    #  rows 32..63 (out half1 = x1*sin + x2*cos; raw=x2, swap=x1): P=cos, Q=sin
    P64 = np.concatenate([cos.T, cos.T], axis=0)                    # [64, T]
    Q64 = np.concatenate([-sin.T, sin.T], axis=0)                   # [64, T]
    P128 = np.concatenate([P64, P64], axis=0).astype(np.float32)
    Q128 = np.concatenate([Q64, Q64], axis=0).astype(np.float32)
    return np.ascontiguousarray(P128), np.ascontiguousarray(Q128)


def make_core_inputs(x, qkv_w, qkv_b, out_w):
    """Build the per-core input maps for the 8-way head-parallel kernel."""
    x = np.asarray(x, dtype=np.float32)
    qkv_w = np.asarray(qkv_w, dtype=np.float32)
    qkv_b = np.asarray(qkv_b, dtype=np.float32)
    out_w = np.asarray(out_w, dtype=np.float32)
    if np.max(np.abs(qkv_b)) != 0.0:
        raise NotImplementedError("kernel assumes qkv_b == 0 (spec fill: zeros)")

    xT = np.ascontiguousarray(x.reshape(BT, D).T)
    ropeP, ropeQ = _rope_tables()
    deint = np.concatenate([np.arange(0, HD, 2), np.arange(1, HD, 2)])  # [64]
    # maskb[:, 384 - c0 : 512] = [zeros(c0) | tri(128)]; tri valid: col >= row
    maskb = np.concatenate(
        [np.zeros((128, 384), dtype=np.float32),
         np.triu(np.ones((128, 128), dtype=np.float32)),
         np.ones((128, 128), dtype=np.float32)],
        axis=1,
    )
    ident = np.concatenate([np.eye(64, dtype=np.float32)] * 2, axis=0)  # [128, 64]
    onescol = np.ones((128, NKB), dtype=np.float32)

    in_maps = []
    for c in range(NCORES):
        cols = []
        for sect, perm in ((0, deint), (1, deint), (2, np.arange(HD))):
            for l in range(HLOC):
                g = HLOC * c + l
                cols.append(sect * D + g * HD + perm)
        cols = np.concatenate(cols)
        w_core = np.ascontiguousarray(qkv_w[:, cols])
        wo_core = np.ascontiguousarray(out_w[c * 128:(c + 1) * 128, :])
        in_maps.append({
            "xT": xT,
            "w": w_core,
            "ropeP": ropeP,
            "ropeQ": ropeQ,
            "maskb": maskb[:, :640],
            "ident": ident,
            "onescol": onescol,
            "wo": wo_core,
        })
    return in_maps


_NC_CACHE = None


def kernel(x, qkv_w, qkv_b, out_w, out_b):
    global _NC_CACHE
    if _NC_CACHE is None:
        _NC_CACHE = build_nc()
    nc = _NC_CACHE
    in_maps = make_core_inputs(x, qkv_w, qkv_b, out_w)
    trace = bool(os.environ.get("ATTN_KERNEL_TRACE"))
    res = run_bass_kernel_spmd(
        nc, in_maps, core_ids=list(range(NCORES)), trace=trace,
    )
    kernel.last_results = res
    y = res.results[0]["y"].astype(np.float64)
    for c in range(1, NCORES):
        y = y + res.results[c]["y"].astype(np.float64)
    y = y + np.asarray(out_b, dtype=np.float64)[None, :]
    return np.ascontiguousarray(y.reshape(B, T, D).astype(np.float32))



# revision 11
# speedup vs baseline: 1.2913x; 1.2913x over previous
"""Causal self-attention (RoPE, 16 heads, D=1024, B=2, T=2048) on 8 TRN2 NeuronCores.

Tensor-parallel over heads (2 heads/core), all matmul operands in bf16
(PSUM accumulation stays fp32; correctness gate is rel_err < 2e-2 and this
lands ~5e-3). Per core:

  - qkv projection: q,k via w-stationary matmuls (hd on partitions, ready for
    rope + scores); v via xt-stationary matmuls producing V^T [tokens, hd]
    directly in PSUM -- no PE transposes at all.
  - rope on DVE (swap-halves via Pool SBUF-SBUF DMA).
  - attention in S^T layout: keys on PSUM partitions, processed in kb-PAIR
    tiles [128, 2, 512] per head so each Exp activation covers 1024
    elems/partition; a ones-row in the V stationary makes PSUM row 0 the
    softmax denominator.
  - output projection from normalized aTb chunks against this core's out_w
    rows; host sums the 8 bf16 partials in fp32.

Scheduling: the S->exp->attnV pipeline is prefetched one pair ahead and the
gap the scalar-engine exp leaves on the tensor engine is filled with an
interleaved stream of later-batch qkv work and output-projection pieces, so
the PE stays continuously busy (TRN2 halves the PE clock when it idles).
Input DMAs are split per-128-row chunk across the SP queue (xT) and the
Activation queue (weights/tables) so the first matmul issues ~1us in.
"""

import os
from collections import deque

import numpy as np

import concourse.mybir as mybir
import concourse.tile as tile
from concourse import bacc
from concourse.bass_utils import run_bass_kernel_spmd

F32 = mybir.dt.float32
BF16 = mybir.dt.bfloat16
EXP = mybir.ActivationFunctionType.Exp

D = 1024
H = 16
HD = 64
B = 2
T = 2048
BT = B * T            # 4096
NCORES = 8
HLOC = H // NCORES    # 2 heads per core
NDC = D // 128        # 8 contraction chunks for the qkv projection
TBW = 512             # qkv token-block width
NTBB = T // TBW       # 4 token blocks per batch
NKB = T // 128        # 16 key blocks per (b, h)
NQ = T // 512         # 4 query super-blocks per (b, h)
SCALE = float(1.0 / np.sqrt(HD))
PE_NS = 0.417         # ns per moving column at full clock
EXP_NS = 1180.0       # est. scalar exp time per pair tile


def build_nc():
    nc = bacc.Bacc("TRN2", debug=False)

    xT = nc.dram_tensor("xT", [D, BT], BF16, kind="ExternalInput")
    w = nc.dram_tensor("w", [D, 2 * HLOC * HD], BF16, kind="ExternalInput")
    wv = nc.dram_tensor("wv", [D, HLOC * HD], BF16, kind="ExternalInput")
    ropeP = nc.dram_tensor("ropeP", [128, T], BF16, kind="ExternalInput")
    ropeQ = nc.dram_tensor("ropeQ", [128, T], BF16, kind="ExternalInput")
    tri = nc.dram_tensor("tri", [128, 128], BF16, kind="ExternalInput")
    onesc = nc.dram_tensor("onesc", [128, NKB * HLOC], BF16, kind="ExternalInput")
    wo = nc.dram_tensor("wo", [128, D], BF16, kind="ExternalInput")
    y = nc.dram_tensor("y", [BT, D], BF16, kind="ExternalOutput")

    with tile.TileContext(nc) as tc:
        with (
            tc.tile_pool(name="const", bufs=1) as const,
            tc.tile_pool(name="big", bufs=1) as big,
            tc.tile_pool(name="xt", bufs=3) as xt_pool,
            tc.tile_pool(name="raw", bufs=2) as raw_pool,
            tc.tile_pool(name="gsw", bufs=2) as gsw_pool,
            tc.tile_pool(name="rtmp", bufs=2) as rtmp_pool,
            tc.tile_pool(name="p_sb", bufs=3) as p_pool,
            tc.tile_pool(name="r_sb", bufs=2) as r_pool,
            tc.tile_pool(name="rb_sb", bufs=2) as rb_pool,
            tc.tile_pool(name="aTb", bufs=4) as aT_pool,
            tc.tile_pool(name="y_sb", bufs=3) as y_pool,
            tc.tile_pool(name="aux_ps", bufs=2, space="PSUM") as aux_psum,
            tc.tile_pool(name="s_ps", bufs=2, space="PSUM") as s_psum,
            tc.tile_pool(name="o_ps", bufs=2, space="PSUM") as o_psum,
        ):
            # ---- startup DMAs. w and the first xT block interleave on the
            # sync queue (first matmul needs w_dc0+xt_dc0 only); rope tables
            # and small consts ride the scalar queue, which must free up fast
            # because scalar compute gates the rope -> attention pipeline.
            w_sb = const.tile([128, NDC, 2 * HLOC * HD], BF16)
            wv_sb = const.tile([128, NDC, HLOC * HD], BF16)
            xt00 = xt_pool.tile([128, NDC, TBW], BF16, name="xt00", tag="xt")
            for dc in range(NDC):
                nc.sync.dma_start(w_sb[:, dc, :], w[dc * 128:(dc + 1) * 128, :])
                nc.sync.dma_start(xt00[:, dc, :], xT[dc * 128:(dc + 1) * 128, 0:TBW])
            for dc in range(NDC):
                nc.sync.dma_start(wv_sb[:, dc, :], wv[dc * 128:(dc + 1) * 128, :])
            P_sb = const.tile([128, T], BF16)
            nc.scalar.dma_start(P_sb[:], ropeP[:])
            Q_sb = const.tile([128, T], BF16)
            nc.scalar.dma_start(Q_sb[:], ropeQ[:])
            tri_sb = const.tile([128, 128], BF16)
            nc.scalar.dma_start(tri_sb[:], tri[:])
            ones_sb = const.tile([128, NKB, HLOC], BF16)
            nc.scalar.dma_start(
                ones_sb[:], onesc[:].rearrange("p (k l) -> p k l", k=NKB)
            )
            wo_sb = const.tile([128, D], BF16)
            nc.scalar.dma_start(wo_sb[:], wo[:])

            qrot = big.tile([128, BT], BF16, tag="qrot")
            krot = big.tile([128, BT], BF16, tag="krot")
            # V^T per batch: [keys(128) x kb x head x (64 hd | ones)]
            vsb = [
                big.tile([128, NKB, HLOC, HD + 1], BF16, name=f"v{b}", tag=f"v{b}")
                for b in range(B)
            ]
            for b in range(B):
                nc.gpsimd.dma_start(vsb[b][:, :, :, HD], ones_sb[:])

            # ---------------- phase-1 (qkv+rope) unit generators ------------
            xt_prefetch = {}

            def make_tb_units(b, tb, phasea, xt_pre=None):
                """Fine-grained emission units for one (b, tb) token block.
                Returns a list of (est_pe_ns, thunk). phasea=True puts the
                PSUM drains on the scalar engine (idle before attention)."""
                state = {}
                if xt_pre is not None:
                    state["xt"] = xt_pre

                def xt_dma():
                    if "xt" in state:
                        return
                    xt_t = xt_pool.tile(
                        [128, NDC, TBW], BF16, name=f"xt{b}{tb}", tag="xt"
                    )
                    col0 = b * T + tb * TBW
                    for dc in range(NDC):
                        nc.sync.dma_start(
                            xt_t[:, dc, :],
                            xT[dc * 128:(dc + 1) * 128, col0:col0 + TBW],
                        )
                    state["xt"] = xt_t

                xt_prefetch[(b, tb)] = xt_dma

                def get_raws():
                    if "raws" not in state:
                        state["raws"] = [
                            raw_pool.tile(
                                [128, TBW], BF16, name=f"raw{b}{tb}{ft}", tag="raw"
                            )
                            for ft in range(2)
                        ]
                    return state["raws"]

                def qk_half(ft, half):
                    if "ps" not in state:
                        state["ps"] = {}
                    if half == 0:
                        state["ps"][ft] = aux_psum.tile(
                            [128, TBW], F32, name=f"qkps{b}{tb}{ft}", tag="aux"
                        )
                    ps = state["ps"][ft]
                    xt_t = state["xt"]
                    for dc in range(half * 4, half * 4 + 4):
                        nc.tensor.matmul(
                            ps[:],
                            w_sb[:, dc, ft * 128:(ft + 1) * 128],
                            xt_t[:, dc, :],
                            start=(dc == 0),
                            stop=(dc == NDC - 1),
                        )
                    if half == 1:
                        dst = get_raws()[ft][:]
                        if phasea:
                            nc.scalar.copy(dst, ps[:])
                        else:
                            nc.vector.tensor_copy(dst, ps[:])

                def rope(which):
                    raw = get_raws()[which]
                    rot = qrot if which == 0 else krot
                    cs = slice(tb * TBW, (tb + 1) * TBW)
                    gsw = gsw_pool.tile(
                        [128, TBW], BF16, name=f"gsw{b}{which}{tb}", tag="gsw"
                    )
                    for l in range(HLOC):
                        p0 = l * 64
                        nc.gpsimd.dma_start(gsw[p0:p0 + 32, :], raw[p0 + 32:p0 + 64, :])
                        nc.gpsimd.dma_start(gsw[p0 + 32:p0 + 64, :], raw[p0:p0 + 32, :])
                    t1 = rtmp_pool.tile(
                        [128, TBW], BF16, name=f"rt{b}{which}{tb}", tag="rt"
                    )
                    nc.vector.tensor_mul(t1[:], raw[:], P_sb[:, cs])
                    nc.vector.tensor_mul(gsw[:], gsw[:], Q_sb[:, cs])
                    nc.vector.tensor_add(
                        rot[:, b * T + tb * TBW:b * T + (tb + 1) * TBW], t1[:], gsw[:]
                    )

                def v_block(tk):
                    xt_t = state["xt"]
                    kb = tb * 4 + tk
                    vp = aux_psum.tile([128, TBW], F32, name=f"vp{b}{kb}", tag="aux")
                    for dc in range(NDC):
                        nc.tensor.matmul(
                            vp[:, 0:HLOC * HD],
                            xt_t[:, dc, tk * 128:(tk + 1) * 128],
                            wv_sb[:, dc, :],
                            start=(dc == 0),
                            stop=(dc == NDC - 1),
                        )
                    src = vp[:, 0:HLOC * HD].rearrange("p (l h) -> p l h", l=HLOC)
                    nc.vector.tensor_copy(vsb[b][:, kb, :, 0:HD], src)

                units = [(0, xt_dma)]
                for ft in range(2):
                    for half in range(2):
                        units.append((854, lambda ft=ft, h=half: qk_half(ft, h)))
                    units.append((0, lambda which=ft: rope(which)))
                for tk in range(4):
                    units.append((427, lambda tk=tk: v_block(tk)))
                return units

            # ---------------- output projection pieces ----------------------
            def outproj_piece(aTb, row_base, i):
                row0 = row_base + i * 128
                ysb = y_pool.tile([128, 2, TBW], BF16, name=f"ysb{row0}", tag="ysb")
                for nb in range(2):
                    yps = aux_psum.tile(
                        [128, TBW], F32, name=f"yps{row0}{nb}", tag="aux"
                    )
                    nc.tensor.matmul(
                        yps[:],
                        aTb[:, i * 128:(i + 1) * 128],
                        wo_sb[:, nb * TBW:(nb + 1) * TBW],
                        start=True,
                        stop=True,
                    )
                    if nb == 0:
                        nc.vector.tensor_copy(ysb[:, nb, :], yps[:])
                    else:
                        nc.scalar.copy(ysb[:, nb, :], yps[:])
                nc.sync.dma_start(
                    y[row0:row0 + 128, :], ysb[:].rearrange("p a t -> p (a t)")
                )

            # ---------------- filler management ------------------------------
            filler = deque()          # (est_ns, thunk)
            markers = {}              # key -> remaining unit count

            def add_units(key, units):
                markers[key] = markers.get(key, 0) + len(units)
                for est, th in units:
                    filler.append((key, est, th))

            def pop_one():
                if not filler:
                    return 0
                key, est, th = filler.popleft()
                th()
                markers[key] -= 1
                return est

            def pop_ns(target):
                got = 0
                while got < target and filler:
                    got += pop_one()

            def flush(key):
                while markers.get(key, 0) > 0:
                    pop_one()

            # ---------------- attention -------------------------------------
            def attention(b, qb_order, extra_ns):
                # sequence of (qb, l, pair) steps
                seq = []
                for qb in qb_order:
                    npair = 2 * (qb + 1)
                    for l in range(HLOC):
                        for p in range(npair):
                            seq.append((qb, l, p, p == 0, p == npair - 1))

                opss = {}
                sps_tiles = {}
                pts = {}

                def s_pair(i):
                    qb, l, p, first, last = seq[i]
                    p0 = l * 64
                    sps = s_psum.tile(
                        [128, 2, TBW], F32, name=f"sps{b}{qb}{l}{p}", tag="sps"
                    )
                    sps_tiles[i] = sps
                    for j in range(2):
                        kb = 2 * p + j
                        r_off = kb - qb * 4
                        cm = 128 * r_off if r_off >= 0 else 0
                        k0 = b * T + kb * 128
                        q0 = b * T + qb * TBW
                        nc.tensor.matmul(
                            sps[:, j, cm:TBW],
                            krot[p0:p0 + 64, k0:k0 + 128],
                            qrot[p0:p0 + 64, q0 + cm:q0 + TBW],
                            start=True,
                            stop=True,
                        )

                def exp_mask(i):
                    qb, l, p, first, last = seq[i]
                    sps = sps_tiles[i]
                    pt = p_pool.tile(
                        [128, 2, TBW], BF16, name=f"pt{b}{qb}{l}{p}", tag="pt"
                    )
                    pts[i] = pt
                    cms = []
                    for j in range(2):
                        r_off = 2 * p + j - qb * 4
                        cms.append(128 * r_off if r_off > 0 else 0)
                    if cms == [0, 0]:
                        nc.scalar.activation(
                            pt[:].rearrange("p a t -> p (a t)"),
                            sps[:].rearrange("p a t -> p (a t)"),
                            EXP,
                            scale=SCALE,
                        )
                    else:
                        for j in range(2):
                            nc.scalar.activation(
                                pt[:, j, cms[j]:TBW], sps[:, j, cms[j]:TBW],
                                EXP, scale=SCALE,
                            )
                    for j in range(2):
                        kb = 2 * p + j
                        r_off = kb - qb * 4
                        if r_off >= 0:
                            cm = 128 * r_off
                            nc.vector.tensor_mul(
                                pt[:, j, cm:cm + 128], pt[:, j, cm:cm + 128], tri_sb[:]
                            )

                def attn_v(i):
                    qb, l, p, first, last = seq[i]
                    nkb = 4 * (qb + 1)
                    ops = opss[(qb, l)]
                    pt = pts.pop(i)
                    for j in range(2):
                        kb = 2 * p + j
                        r_off = kb - qb * 4
                        cm = 128 * r_off if r_off >= 0 else 0
                        nc.tensor.matmul(
                            ops[:, cm:TBW],
                            vsb[b][:, kb, l, :],
                            pt[:, j, cm:TBW],
                            start=(kb == 0),
                            stop=(kb == nkb - 1),
                        )
                    del sps_tiles[i]

                def denorm(qb, l):
                    ops = opss.pop((qb, l))
                    aTb = state_aTb[qb]
                    d_sb = r_pool.tile([1, TBW], F32, tag="d")
                    nc.vector.tensor_copy(d_sb[:], ops[HD:HD + 1, :])
                    r_sb = r_pool.tile([1, TBW], F32, tag="r")
                    nc.vector.reciprocal_approx_fast(r_sb[:], d_sb[:])
                    rb_sb = rb_pool.tile([64, TBW], F32, tag="rb")
                    nc.gpsimd.partition_broadcast(rb_sb[:], r_sb[:])
                    nc.vector.tensor_mul(
                        aTb[l * 64:(l + 1) * 64, :], ops[0:HD, :], rb_sb[:]
                    )

                state_aTb = {}

                def step_cols(i):
                    qb, l, p, first, last = seq[i]
                    cols = 0
                    for j in range(2):
                        r_off = 2 * p + j - qb * 4
                        cm = 128 * r_off if r_off >= 0 else 0
                        cols += TBW - cm
                    return cols

                def prologue(i):
                    # must run BEFORE s_pair(i) is emitted: the flush puts the
                    # qkv units this qb depends on ahead of it in the in-order
                    # PE queue.
                    qb, l, p, first, last = seq[i]
                    if first and l == 0:
                        for tbn in range(qb + 1):
                            flush(("tb", b, tbn))
                        pos = qb_order.index(qb)
                        if pos + 1 < len(qb_order):
                            nxt = qb_order[pos + 1]
                            for tbn in range(nxt + 1):
                                if (b, tbn) in xt_prefetch:
                                    xt_prefetch[(b, tbn)]()
                        state_aTb[qb] = aT_pool.tile(
                            [128, TBW], BF16, name=f"aTb{b}{qb}", tag="aTb"
                        )
                    if first:
                        opss[(qb, l)] = o_psum.tile(
                            [HD + 1, TBW], F32, name=f"ops{b}{qb}{l}", tag="ops"
                        )

                prologue(0)
                s_pair(0)
                exp_mask(0)
                for i, (qb, l, p, first, last) in enumerate(seq):
                    pe_ns = step_cols(i) * PE_NS
                    if i + 1 < len(seq):
                        prologue(i + 1)
                        s_pair(i + 1)
                        exp_mask(i + 1)
                    pop_ns(max(0.0, EXP_NS - pe_ns) + extra_ns)
                    attn_v(i)
                    if last:
                        denorm(qb, l)
                        if l == HLOC - 1:
                            aTb = state_aTb.pop(qb)
                            row_base = b * T + qb * TBW
                            units = [
                                (427, lambda aTb=aTb, rb=row_base, i2=i2:
                                 outproj_piece(aTb, rb, i2))
                                for i2 in range(4)
                            ]
                            add_units(("op", b, qb), units)

            # ---------------- main schedule ----------------------------------
            # batch 0, token block 0 emitted directly (lead-in).
            for est, th in make_tb_units(0, 0, phasea=True, xt_pre=xt00):
                th()
            # later token blocks become filler, keyed by the attention qb that
            # needs them: b0 attention qb_i needs b0 tb_i; b1 likewise.
            for tb in range(1, NTBB):
                add_units(("tb", 0, tb), make_tb_units(0, tb, phasea=False))
            for tb in range(NTBB):
                add_units(("tb", 1, tb), make_tb_units(1, tb, phasea=False))

            attention(0, [0, 1, 2, 3], 250.0)
            attention(1, [3, 2, 1, 0], 500.0)
            while filler:
                pop_one()

    nc.finalize()
    return nc


def _rope_tables():
    inv_freq = 1.0 / (10000.0 ** (np.arange(0, HD, 2, dtype=np.float32) / HD))
    t = np.arange(T, dtype=np.float32)
    freqs = t[:, None] * inv_freq[None, :]                          # [T, 32]
    rope = np.concatenate([np.sin(freqs), np.cos(freqs)], axis=-1)  # [T, 64]
    sin = rope[:, ::2]    # [T, 32]  (reference's "sin")
    cos = rope[:, 1::2]   # [T, 32]  (reference's "cos")
    # rot = raw * P + swap_halves(raw) * Q  with raw rows [x1(32) ; x2(32)]:
    #  rows 0..31  (out half0 = x1*cos - x2*sin; raw=x1, swap=x2): P=cos, Q=-sin
    #  rows 32..63 (out half1 = x1*sin + x2*cos; raw=x2, swap=x1): P=cos, Q=sin
    P64 = np.concatenate([cos.T, cos.T], axis=0)                    # [64, T]
    Q64 = np.concatenate([-sin.T, sin.T], axis=0)                   # [64, T]
    P128 = np.concatenate([P64, P64], axis=0).astype(np.float32)
    Q128 = np.concatenate([Q64, Q64], axis=0).astype(np.float32)
    return np.ascontiguousarray(P128), np.ascontiguousarray(Q128)


def make_core_inputs(x, qkv_w, qkv_b, out_w):
    """Build the per-core input maps for the 8-way head-parallel kernel."""
    import ml_dtypes

    bf16 = ml_dtypes.bfloat16
    x = np.asarray(x, dtype=np.float32)
    qkv_w = np.asarray(qkv_w, dtype=np.float32)
    qkv_b = np.asarray(qkv_b, dtype=np.float32)
    out_w = np.asarray(out_w, dtype=np.float32)
    if np.max(np.abs(qkv_b)) != 0.0:
        raise NotImplementedError("kernel assumes qkv_b == 0 (spec fill: zeros)")

    xT = np.ascontiguousarray(x.reshape(BT, D).T.astype(bf16))
    ropeP, ropeQ = _rope_tables()
    ropeP = ropeP.astype(bf16)
    ropeQ = ropeQ.astype(bf16)
    deint = np.concatenate([np.arange(0, HD, 2), np.arange(1, HD, 2)])  # [64]
    tri = np.triu(np.ones((128, 128), dtype=np.float32)).astype(bf16)
    onesc = np.ones((128, NKB * HLOC), dtype=np.float32).astype(bf16)

    in_maps = []
    for c in range(NCORES):
        cols = []
        for sect in (0, 1):
            for l in range(HLOC):
                g = HLOC * c + l
                cols.append(sect * D + g * HD + deint)
        cols = np.concatenate(cols)
        w_core = np.ascontiguousarray(qkv_w[:, cols].astype(bf16))
        vcols = np.concatenate(
            [2 * D + (HLOC * c + l) * HD + np.arange(HD) for l in range(HLOC)]
        )
        wv_core = np.ascontiguousarray(qkv_w[:, vcols].astype(bf16))
        wo_core = np.ascontiguousarray(out_w[c * 128:(c + 1) * 128, :].astype(bf16))
        in_maps.append({
            "xT": xT,
            "w": w_core,
            "wv": wv_core,
            "ropeP": ropeP,
            "ropeQ": ropeQ,
            "tri": tri,
            "onesc": onesc,
            "wo": wo_core,
        })
    return in_maps


_NC_CACHE = None


def kernel(x, qkv_w, qkv_b, out_w, out_b):
    global _NC_CACHE
    if _NC_CACHE is None:
        _NC_CACHE = build_nc()
    nc = _NC_CACHE
    in_maps = make_core_inputs(x, qkv_w, qkv_b, out_w)
    trace = bool(os.environ.get("ATTN_KERNEL_TRACE"))
    res = run_bass_kernel_spmd(
        nc, in_maps, core_ids=list(range(NCORES)), trace=trace,
    )
    kernel.last_results = res
    y = res.results[0]["y"].astype(np.float64)
    for c in range(1, NCORES):
        y = y + res.results[c]["y"].astype(np.float64)
    y = y + np.asarray(out_b, dtype=np.float64)[None, :]
    return np.ascontiguousarray(y.reshape(B, T, D).astype(np.float32))
